# revision 15
# baseline (speedup 1.0000x reference)
"""Trainium2 Bass kernel for a dense transformer block (B=2, T=2048, C=1024,
NH=16, HD=64, FF=4x), distributed over 8 NeuronCores.

Sharding: data-parallel over batch (2 groups of 4 cores) x tensor-parallel over
heads within a group (4 heads/core, Megatron-style), with sequence-parallel FFN:
the attention output partials are ReduceScattered over the T axis inside each
group, so each core runs LN2+FFN on its own 512 rows and returns them. The host
assembles the full [B, T, C] output. Matmul operands are bf16 (full PE rate,
1 cycle/row incl. 64-partition score matmuls and stationary loads); LayerNorm
and softmax statistics plus all PSUM accumulation stay in fp32.

LN gains are folded into the weights host-side (exact algebra):
  xn = g*z + be  (z = (x-mean)/std)  =>  xn @ W = z @ (g*W) + be @ W
so the chip computes z only; the be@W terms become small per-column biases.

DMA queue assignment (each engine sequencer issues its own in-order queue):
  sync    - x tiles, xres residuals, y output
  scalar  - attention weights/biases (wq/wk/wv/wo/bqk/bv/b1/b2)
  vector  - ReduceScatter staging writes (producer is vector anyway)
  gpsimd  - ReduceScatter result reads + FFN weight streaming (idle engine,
            so FFN weights prefetch during late attention)
"""

import contextlib
import os
import sys
import types

import ml_dtypes
import numpy as np

# --- NTFF profile hook shim (tracing support; harmless when unused) ---------
def _install_ntff_hook_shim():
    if "antenv.axon_hooks" in sys.modules:
        return
    try:
        import antenv
        import trn_agent_boot.trn_boot as tb

        mod = types.ModuleType("antenv.axon_hooks")
        holder = [None]
        mod.set_axon_ntff_profile_hook = lambda h: holder.__setitem__(0, h)
        mod.get_axon_ntff_profile_hook = lambda: holder[0]
        sys.modules["antenv.axon_hooks"] = mod
        antenv.axon_hooks = mod
        if os.path.exists("/opt/axon/libaxon_pjrt.so"):
            mod.set_axon_ntff_profile_hook(
                tb._ntff_profile_via_ctypes("/opt/axon/libaxon_pjrt.so")
            )
    except Exception:
        pass


_install_ntff_hook_shim()

import concourse.bass as bass
import concourse.mybir as mybir
import concourse.tile as tile
from concourse import bacc
from concourse import bass_utils
from concourse.masks import make_identity

# Problem shape (hardcoded per contest rules).
B, T, C, NH, HD = 2, 2048, 1024, 16, 64
FF = 4 * C  # 4096
EPS = 1e-6
P = 128
NCORES = 8
TPG = 4            # cores per batch group
NHL = NH // TPG    # local heads per core = 4
TLOC = T // TPG    # rows per core after ReduceScatter = 512
KO = C // P        # 8 contraction chunks over C
NFT = FF // P      # 32 f-tiles
NTT = T // P       # 16 t-tiles
NTB = T // 512     # 4 t-blocks

DT = mybir.dt.bfloat16   # matmul operand dtype
F32 = mybir.dt.float32
NPDT = ml_dtypes.bfloat16
MASK_NEG = -30000.0

_CACHED_NC = None

# Results of the most recent hardware run (BassKernelResults); lets test
# harnesses read exec_time_ns when BASS_TRACE=1.
LAST_RESULTS = None


def _build_module():
    nc = bacc.Bacc("TRN2", target_bir_lowering=False, debug=False,
                   num_devices=NCORES)

    x_in = nc.dram_tensor("xh", [T, C], DT, kind="ExternalInput").ap()
    xres_in = nc.dram_tensor("xres", [TLOC, C], F32, kind="ExternalInput").ap()
    wq_in = nc.dram_tensor("wq", [P, KO, NHL * HD], DT, kind="ExternalInput").ap()
    wk_in = nc.dram_tensor("wk", [P, KO, NHL * HD], DT, kind="ExternalInput").ap()
    wv_in = nc.dram_tensor("wv", [P, KO, NHL * HD], DT, kind="ExternalInput").ap()
    bqk_in = nc.dram_tensor("bqk", [P, 4], F32, kind="ExternalInput").ap()
    bv_in = nc.dram_tensor("bv", [1, NHL * HD], F32, kind="ExternalInput").ap()
    wo_in = nc.dram_tensor("wo", [P, 2, C], DT, kind="ExternalInput").ap()
    bo_in = nc.dram_tensor("bo", [1, C], F32, kind="ExternalInput").ap()
    w1_in = nc.dram_tensor("w1", [P, NFT, KO, P], DT, kind="ExternalInput").ap()
    b1_in = nc.dram_tensor("b1p", [P, NFT], F32, kind="ExternalInput").ap()
    w2_in = nc.dram_tensor("w2", [P, NFT, C], DT, kind="ExternalInput").ap()
    b2_in = nc.dram_tensor("b2", [1, C], F32, kind="ExternalInput").ap()
    y_out = nc.dram_tensor("y", [TLOC, C], F32, kind="ExternalOutput").ap()

    with tile.TileContext(nc) as tc:
        _emit(nc, tc, x_in, xres_in, wq_in, wk_in, wv_in, bqk_in, bv_in,
              wo_in, bo_in, w1_in, b1_in, w2_in, b2_in, y_out)
    nc.compile()
    return nc


def _layernorm_z(nc, stats, xt, z_out):
    """z = (x - mean(x)) / unbiased_std(x), rows on partitions.

    Rsqrt keeps the scalar engine on a single activation table (the
    reference's +EPS on std is a 1e-6 relative difference, far below
    tolerance).  xt: [P, C] fp32 SBUF tile; z_out: [P, C] DT tile."""
    s6 = stats.tile([P, 2, 6], F32, tag="bn6")
    nc.vector.bn_stats(s6[:, 0, :], xt[:, 0:C // 2])
    nc.vector.bn_stats(s6[:, 1, :], xt[:, C // 2:C])
    mv = stats.tile([P, 2], F32, tag="bnmv")
    nc.vector.bn_aggr(mv[:], s6[:])
    std = stats.tile([P, 1], F32, tag="bnstd")
    nc.scalar.activation(std[:], mv[:, 1:2],
                         mybir.ActivationFunctionType.Sqrt,
                         scale=float(C) / float(C - 1))
    rstd = stats.tile([P, 1], F32, tag="bnrstd")
    nc.vector.reciprocal(rstd[:], std[:])
    nc.vector.tensor_scalar(z_out[:], xt[:], mv[:, 0:1], rstd[:],
                            mybir.AluOpType.subtract, mybir.AluOpType.mult)


def _emit(nc, tc, x_in, xres_in, wq_in, wk_in, wv_in, bqk_in, bv_in,
          wo_in, bo_in, w1_in, b1_in, w2_in, b2_in, y_out):
    ctx = contextlib.ExitStack()
    # persistent pools (whole kernel)
    fp = ctx.enter_context(tc.tile_pool(name="fixed", bufs=1))
    stats = ctx.enter_context(tc.tile_pool(name="stats", bufs=6))
    ztiles = ctx.enter_context(tc.tile_pool(name="ztiles", bufs=2))
    dram = ctx.enter_context(tc.tile_pool(name="dram", bufs=1, space="DRAM"))

    # --- persistent constants -----------------------------------------------
    ident = fp.tile([P, P], DT)
    make_identity(nc, ident[:])
    ones1 = fp.tile([1, P], F32)
    nc.vector.memset(ones1[:], 1.0)
    ones1r = fp.tile([1, P], DT)
    nc.vector.tensor_copy(ones1r[:], ones1[:])
    b1p = fp.tile([P, NFT], F32)
    b2 = fp.tile([1, C], F32)
    b2r = fp.tile([1, C], DT)
    w2keep = fp.tile([P, NFT, C], DT)

    rs_ins = [dram.tile([512, C], DT, name=f"rsin{j}") for j in range(NTB)]
    rs_outs = [dram.tile([P, C], DT, name=f"rsout{j}") for j in range(NTB)]

    # Tiny warmup ReduceScatter issued during P1: the first collective pays
    # ~25us of channel setup that would otherwise land on band 0's RS.
    warm_in = dram.tile([TPG, P], F32, name="warmin")
    warm_out = dram.tile([1, P], F32, name="warmout")
    wtile = fp.tile([TPG, P], F32)
    nc.vector.memset(wtile[:], 0.0)
    nc.sync.dma_start(warm_in[:], wtile[:])
    nc.gpsimd.collective_compute(
        "ReduceScatter", mybir.AluOpType.add,
        replica_groups=[[0, 1, 2, 3], [4, 5, 6, 7]],
        ins=[warm_in.opt()], outs=[warm_out.opt()],
    )

    # attention-scope pools: released after P4
    actx = contextlib.ExitStack()
    fpa = actx.enter_context(tc.tile_pool(name="fixeda", bufs=1))
    abig = actx.enter_context(tc.tile_pool(name="abig", bufs=1))

    zero512 = fpa.tile([P, 512], F32)
    nc.vector.memset(zero512[:], 0.0)
    masks = fpa.tile([P, 512], F32)
    # keep score where (t_rel - s_rel) >= 0 else MASK_NEG; diagonal blocks are
    # emitted in truncated coordinates so the k=0 mask serves all of them.
    nc.gpsimd.affine_select(
        out=masks[:], in_=zero512[:],
        compare_op=mybir.AluOpType.is_ge, fill=MASK_NEG,
        base=0, channel_multiplier=-1, pattern=[[1, 512]],
    )

    qctx = contextlib.ExitStack()
    qkvw = qctx.enter_context(tc.tile_pool(name="qkvw", bufs=1))
    # weight/constant DMAs ride the scalar queue so the x tiles (sync queue)
    # land first; order matches first use on chip.
    bqk = qkvw.tile([P, 4], F32)
    nc.scalar.dma_start(bqk[:], bqk_in[:])
    bv = qkvw.tile([1, NHL * HD], F32)
    nc.scalar.dma_start(bv[:], bv_in[:])
    nc.scalar.dma_start(b1p[:], b1_in[:])
    nc.scalar.dma_start(b2[:], b2_in[:])
    nc.vector.tensor_copy(b2r[:], b2[:])
    bv_r = qkvw.tile([1, NHL * HD], DT)
    nc.vector.tensor_copy(bv_r[:], bv[:])
    # wq/wk/wv/wo DMAs are issued from inside the P1 loop (below) so the
    # first x tiles own the DMA rings at startup.
    wq = qkvw.tile([P, KO, NHL * HD], DT)
    wk = qkvw.tile([P, KO, NHL * HD], DT)
    wv = qkvw.tile([P, KO, NHL * HD], DT)
    wo = fpa.tile([P, 2, C], DT)

    qT2 = abig.tile([P, 2, T], DT)
    kT2 = abig.tile([P, 2, T], DT)
    v_sb = abig.tile([P, NTT, NHL, HD + 1], DT)
    ones_c = fpa.tile([P, 1], F32)
    nc.vector.memset(ones_c[:], 1.0)
    # esel: [33, P] selector; row 0 -> out partitions 0:64, row 32 ->
    # 64:128 (SBUF partition bases must be 0/32/64, so a Z pair lives on
    # rows 0 and 32).  One PE op broadcasts both 1/Z rows of a head pair.
    esel = fpa.tile([33, P], DT)
    nc.vector.memset(esel[:], 0.0)
    nc.vector.memset(esel[0:1, 0:HD], 1.0)
    nc.vector.memset(esel[32:33, HD:P], 1.0)
    nc.vector.tensor_copy(
        v_sb[:, :, :, HD:HD + 1],
        ones_c[:, :, None, None].to_broadcast((P, NTT, NHL, 1)))

    # --- P1+P2: per 512-row block: LN1, transpose, QKV ----------------------
    with tc.tile_pool(name="xnTp", bufs=2) as xnTp, \
         tc.tile_pool(name="xtiles", bufs=3) as xtiles, \
         tc.tile_pool(name="pst1", bufs=3, space="PSUM") as pst1, \
         tc.tile_pool(name="psqk", bufs=3, space="PSUM") as psqk:
        for tb in range(NTB):
            xnT = xnTp.tile([P, KO, 512], DT, tag="xnT")
            for tt4 in range(4):
                tt = 4 * tb + tt4
                xt = xtiles.tile([P, C], DT, tag="x")
                nc.sync.dma_start(xt[:, 0:C // 2],
                                  x_in[tt * P:(tt + 1) * P, 0:C // 2])
                nc.sync.dma_start(xt[:, C // 2:C],
                                  x_in[tt * P:(tt + 1) * P, C // 2:C])
                z = ztiles.tile([P, C], DT, tag="z")
                _layernorm_z(nc, stats, xt, z)
                if tt == 1:
                    nc.scalar.dma_start(wq[:], wq_in[:])
                elif tt == 2:
                    nc.scalar.dma_start(wk[:], wk_in[:])
                elif tt == 3:
                    nc.scalar.dma_start(wv[:], wv_in[:])
                elif tt == 5:
                    nc.scalar.dma_start(wo[:], wo_in[:])
                for cg in range(2):
                    tp = pst1.tile([P, 512], DT, tag="tp")
                    for c4 in range(4):
                        ci = 4 * cg + c4
                        nc.tensor.transpose(tp[:, c4 * P:(c4 + 1) * P],
                                            z[:, ci * P:(ci + 1) * P],
                                            ident[:])
                    nc.vector.tensor_copy(
                        xnT[:, 4 * cg:4 * cg + 4, tt4 * P:(tt4 + 1) * P],
                        tp.rearrange("p (c t) -> p c t", c=4))
            for pp in range(2):
                for dst, w, bcol in ((qT2, wq, pp), (kT2, wk, 2 + pp)):
                    ps = psqk.tile([P, 512], F32, tag="qk")
                    for ko in range(KO):
                        nc.tensor.matmul(
                            ps[:], w[:, ko, pp * P:(pp + 1) * P],
                            xnT[:, ko, :],
                            start=(ko == 0), stop=(ko == KO - 1))
                    nc.vector.tensor_scalar_add(
                        dst[:, pp, tb * 512:(tb + 1) * 512], ps[:],
                        bqk[:, bcol:bcol + 1])
            for tt4 in range(4):
                tt = 4 * tb + tt4
                ps = psqk.tile([P, NHL * HD], F32, tag="qk")
                nc.tensor.matmul(ps[:, 0:NHL * HD], ones1r[0:1, :],
                                 bv_r[0:1, :],
                                 start=True, stop=False, skip_group_check=True)
                for ko in range(KO):
                    nc.tensor.matmul(
                        ps[:, 0:NHL * HD],
                        xnT[:, ko, tt4 * P:(tt4 + 1) * P], wv[:, ko, :],
                        start=False, stop=(ko == KO - 1),
                        skip_group_check=True)
                for h in range(NHL):
                    nc.vector.tensor_copy(v_sb[:, tt, h, 0:HD],
                                          ps[:, h * HD:(h + 1) * HD])
    qctx.close()

    # --- P3..P7 ---------------------------------------------------------------
    x2 = ctx.enter_context(tc.tile_pool(name="x2p", bufs=1, side="right")).tile(
        [P, TLOC // P, C], F32)
    xn2T = ctx.enter_context(
        tc.tile_pool(name="xn2Tp", bufs=1, side="right")).tile(
        [P, KO, TLOC], DT)

    with tc.tile_pool(name="ptp", bufs=4) as ptp, \
         tc.tile_pool(name="mskp", bufs=2) as mskp, \
         tc.tile_pool(name="rzp", bufs=3) as rzp, \
         tc.tile_pool(name="bandp", bufs=2) as bandp, \
         tc.tile_pool(name="rstage", bufs=2) as rstage, \
         tc.tile_pool(name="p6", bufs=2) as p6, \
         tc.tile_pool(name="pst2", bufs=1, space="PSUM") as pst2:

        def emit_p6(j):
            """x2 band j = rs_out_j + (xres+bo); LN2; transpose into xn2T."""
            # rs_out depends on the collective: keep its DMA on the gpsimd
            # queue so no compute queue head-of-line blocks on it.
            x2b = p6.tile([P, C], DT, tag="x2b")
            nc.gpsimd.dma_start(x2b[:], rs_outs[j][:])
            xrt = p6.tile([P, C], F32, tag="xrt")
            nc.sync.dma_start(xrt[:], xres_in[j * P:(j + 1) * P, :])
            nc.vector.tensor_add(x2[:, j, :], x2b[:], xrt[:])
            z2 = ztiles.tile([P, C], DT, tag="z")
            _layernorm_z(nc, stats, x2[:, j, :], z2)
            for cg in range(2):
                tp = pst2.tile([P, 512], DT, tag="tp2")
                for c4 in range(4):
                    ci = 4 * cg + c4
                    nc.tensor.transpose(tp[:, c4 * P:(c4 + 1) * P],
                                        z2[:, ci * P:(ci + 1) * P],
                                        ident[:])
                nc.vector.tensor_copy(
                    xn2T[:, 4 * cg:4 * cg + 4, j * P:(j + 1) * P],
                    tp.rearrange("p (c t) -> p c t", c=4))

        with tc.tile_pool(name="psc", bufs=2, space="PSUM") as pssc, \
             tc.tile_pool(name="psctx", bufs=3, space="PSUM") as psctx, \
             tc.tile_pool(name="pso", bufs=2, space="PSUM") as pso:

            def emit_wo(j, ctxb):
                """Wo partials for band j -> rs_in_j, then its own RS."""
                for tt4 in range(4):
                    stg = rstage.tile([P, C], DT, tag="stg")
                    for cb in range(2):
                        ops_ = pso.tile([P, 512], F32, tag="o")
                        for ch in range(2):
                            nc.tensor.matmul(
                                ops_[:],
                                ctxb[:, ch, tt4 * P:(tt4 + 1) * P],
                                wo[:, ch, cb * 512:(cb + 1) * 512],
                                start=(ch == 0), stop=(ch == 1))
                        nc.vector.tensor_copy(stg[:, cb * 512:(cb + 1) * 512],
                                              ops_[:])
                    nc.scalar.dma_start(rs_ins[j][tt4 * P:(tt4 + 1) * P, :],
                                        stg[:])
                nc.gpsimd.collective_compute(
                    "ReduceScatter", mybir.AluOpType.add,
                    replica_groups=[[0, 1, 2, 3], [4, 5, 6, 7]],
                    ins=[rs_ins[j].opt()], outs=[rs_outs[j].opt()],
                )

            # Bands run in DESCENDING order: band 3 (the largest) finishes
            # first so its ReduceScatter lands long before the FFN needs it;
            # band 0's RS is covered by the first FFN half (bands 2,3).
            pend_wo = None
            for bi, j in enumerate([3, 2, 1, 0]):
                ns = 4 * (j + 1)
                ctxb = bandp.tile([P, 2, 512], DT, tag="ctxb")
                # 1/Z staging: Z rows gathered per head pair, reciprocal
                # batched over 2 partitions so the serial [_,512] sweep is
                # paid twice per band, not 4x.  esel broadcasts both rows of
                # a pair to 64 partitions each in one PE op.
                zrp = [rzp.tile([33, 512], F32, tag=f"zr{p}", name=f"zr{p}")
                       for p in range(2)]
                rzp_ = [rzp.tile([33, 512], DT, tag=f"rz{p}", name=f"rz{p}")
                        for p in range(2)]
                for p in range(2):
                    # rows 1..31 are never written; keep them finite so the
                    # batched reciprocal and the esel matmul stay clean.
                    nc.vector.memset(zrp[p][:], 1.0)

                def norm_pair(pp, cps01):
                    """PE broadcast of a pair's 1/Z + fused psum->sbuf
                    scale into ctxb[:, pp, :]."""
                    zbp = pso.tile([P, 512], F32, tag="o")
                    nc.tensor.matmul(zbp[:], esel[:], rzp_[pp][:],
                                     start=True, stop=True)
                    zb = rzp.tile([P, 512], F32, tag="zb")
                    nc.vector.tensor_copy(zb[:], zbp[:])
                    for sub in range(2):
                        nc.vector.tensor_tensor(
                            ctxb[sub * HD:(sub + 1) * HD, pp, :],
                            cps01[sub][0:HD, :], zb[sub * HD:(sub + 1) * HD, :],
                            mybir.AluOpType.mult)

                norm_q = []
                for h in range(NHL):
                    pp, poff = h // 2, 64 * (h % 2)
                    cps = psctx.tile([HD + 1, 512], F32, tag="ctx")
                    for i in range(ns):
                        k = i - 4 * j
                        off = max(k, 0) * P       # truncated query offset
                        W = 512 - off
                        sps = pssc.tile([P, 512], F32, tag="sc")
                        nc.tensor.matmul(
                            sps[:, 0:W],
                            kT2[poff:poff + HD, pp, i * P:(i + 1) * P],
                            qT2[poff:poff + HD, pp,
                                j * 512 + off:(j + 1) * 512],
                            start=True, stop=True)
                        pT = ptp.tile([P, 512], DT, tag="pT")
                        if k >= 0:
                            ms = mskp.tile([P, 512], DT, tag="ms")
                            nc.vector.scalar_tensor_tensor(
                                ms[:, 0:W], sps[:, 0:W], 0.125, masks[:, 0:W],
                                mybir.AluOpType.mult, mybir.AluOpType.add)
                            nc.scalar.activation(
                                pT[:, 0:W], ms[:, 0:W],
                                mybir.ActivationFunctionType.Exp)
                            # region [off:off+128] sees its last key block
                            # here; later diagonals only touch columns right
                            # of it, so close its accumulation now.
                            nc.tensor.matmul(cps[:, off:off + P],
                                             v_sb[:, i, h, :], pT[:, 0:P],
                                             start=(i == 0), stop=True,
                                             skip_group_check=True)
                            if k < 3:
                                nc.tensor.matmul(cps[:, off + P:512],
                                                 v_sb[:, i, h, :],
                                                 pT[:, P:W],
                                                 start=(i == 0), stop=False,
                                                 skip_group_check=True)
                        else:
                            nc.scalar.activation(
                                pT[:], sps[:],
                                mybir.ActivationFunctionType.Exp, scale=0.125)
                            nc.tensor.matmul(cps[:], v_sb[:, i, h, :], pT[:],
                                             start=(i == 0), stop=False,
                                             skip_group_check=True)
                    zrow = 32 * (h % 2)
                    nc.vector.tensor_copy(zrp[h // 2][zrow:zrow + 1, :],
                                          cps[HD:HD + 1, :])
                    if h % 2 == 1:
                        with nc.allow_low_precision(
                                reason="1/Z rounded to bf16 for PE broadcast"):
                            nc.vector.reciprocal(rzp_[h // 2][:],
                                                 zrp[h // 2][:])
                    norm_q.append(cps)
                    if h == 0 and pend_wo is not None:
                        emit_wo(*pend_wo)
                        pend_wo = None
                    if h == 2:
                        norm_pair(0, norm_q[0:2])
                norm_pair(1, norm_q[2:4])

                if bi < NTB - 1:
                    pend_wo = (j, ctxb)
                else:
                    emit_wo(j, ctxb)
                if j == 2:
                    # RS3 has landed by the time the queues get here.
                    emit_p6(3)
                    # W2 prefetch: issue the whole stream now (no pool
                    # throttling) so it's resident before the FFN starts.
                    for ft in range(NFT):
                        nc.gpsimd.dma_start(w2keep[:, ft, :], w2_in[:, ft, :])
                elif j == 1:
                    emit_p6(2)
            emit_p6(1)

        # --- P7: FFN; th half 0 first (bands 0,1), band-3 P6 in between ------
        # W2 is loaded once into SBUF (w2keep) during half 0 and reused in
        # half 1; W1 streams per half on the gpsimd queue, which at emission
        # time sits just after the band-2 rs_out read, so half-0 weights
        # prefetch during band-3 attention compute.
        with tc.tile_pool(name="w1p", bufs=6) as w1p, \
             tc.tile_pool(name="rp", bufs=2) as rp, \
             tc.tile_pool(name="psh1", bufs=3, space="PSUM") as psh1, \
             tc.tile_pool(name="psh2", bufs=4, space="PSUM") as psh2, \
             tc.tile_pool(name="yp", bufs=2) as yp:

            def ffn_half(th, mid=None):
                h2ps = []
                for tt2 in range(2):
                    for cb in range(2):
                        hp = psh2.tile([P, 512], F32, tag="h2")
                        nc.tensor.matmul(hp[:], ones1r[0:1, :],
                                         b2r[0:1, cb * 512:(cb + 1) * 512],
                                         start=True, stop=False,
                                         skip_group_check=True)
                        h2ps.append(hp)
                for ft in range(NFT):
                    if ft == 10 and mid is not None:
                        mid()
                    w1t = w1p.tile([P, KO, P], DT, tag="w1")
                    nc.gpsimd.dma_start(w1t[:], w1_in[:, ft, :, :])
                    h1 = psh1.tile([P, 256], F32, tag="h1")
                    for ko in range(KO):
                        nc.tensor.matmul(h1[:], w1t[:, ko, :],
                                         xn2T[:, ko, th * 256:(th + 1) * 256],
                                         start=(ko == 0), stop=(ko == KO - 1))
                    rT = rp.tile([P, 256], DT, tag="rT")
                    nc.vector.tensor_scalar(rT[:], h1[:], b1p[:, ft:ft + 1],
                                            0.0, mybir.AluOpType.add,
                                            mybir.AluOpType.max)
                    for tt2 in range(2):
                        for cb in range(2):
                            nc.tensor.matmul(
                                h2ps[2 * tt2 + cb][:],
                                rT[:, tt2 * P:(tt2 + 1) * P],
                                w2keep[:, ft, cb * 512:(cb + 1) * 512],
                                start=False, stop=(ft == NFT - 1),
                                skip_group_check=True)
                for tt2 in range(2):
                    gt = 2 * th + tt2
                    for cb in range(2):
                        yt = yp.tile([P, 512], F32, tag="y")
                        nc.vector.scalar_tensor_tensor(
                            yt[:], h2ps[2 * tt2 + cb][:], 1.0,
                            x2[:, gt, cb * 512:(cb + 1) * 512],
                            mybir.AluOpType.mult, mybir.AluOpType.add)
                        nc.sync.dma_start(
                            y_out[gt * P:(gt + 1) * P,
                                  cb * 512:(cb + 1) * 512],
                            yt[:])

            ffn_half(1, mid=lambda: emit_p6(0))
            ffn_half(0)

    actx.close()
    ctx.close()


def _prep_inputs(x, Wq, Wk, Wv, Wo, bo, W1, b1, W2, b2, g1, be1, g2, be2):
    """Host-side sharding + layout packing. Returns list of 8 in_maps."""
    f32 = np.float32
    x = np.asarray(x, f32)
    Wq, Wk, Wv = (np.asarray(a, f32) for a in (Wq, Wk, Wv))
    Wo, bo = np.asarray(Wo, f32), np.asarray(bo, f32)
    W1, b1, W2, b2 = (np.asarray(a, f32) for a in (W1, b1, W2, b2))
    g1, be1, g2, be2 = (np.asarray(a, np.float64) for a in (g1, be1, g2, be2))

    def pack_qkv(W):  # [NHL, C, HD] g-folded -> [P, KO, NHL*HD]
        Wl = (g1[None, :, None] * W.astype(np.float64)).astype(f32)
        flat = Wl.transpose(1, 0, 2).reshape(C, NHL * HD)   # [c, col]
        return np.ascontiguousarray(flat.reshape(KO, P, NHL * HD)
                                    .transpose(1, 0, 2)).astype(NPDT)

    # W1 folded with g2: [C, FF] -> [P, NFT, KO, P]
    W1f = (g2[:, None] * W1.astype(np.float64)).astype(f32)
    w1_arr = np.ascontiguousarray(
        W1f.reshape(KO, P, NFT, P).transpose(1, 2, 0, 3)).astype(NPDT)
    b1p = (b1.astype(np.float64) + be2 @ W1.astype(np.float64)).astype(f32)
    b1_arr = np.ascontiguousarray(b1p.reshape(NFT, P).T)
    w2_arr = np.ascontiguousarray(
        W2.reshape(NFT, P, C).transpose(1, 0, 2)).astype(NPDT)
    b2_arr = b2.reshape(1, C)
    bo_arr = bo.reshape(1, C)

    in_maps = []
    for core in range(NCORES):
        b, r = divmod(core, TPG)
        hsel = slice(NHL * r, NHL * (r + 1))
        wq_arr = pack_qkv(Wq[hsel])
        wk_arr = pack_qkv(Wk[hsel])
        wv_arr = pack_qkv(Wv[hsel])
        # be1-induced biases (exact): col order = head-major within 256
        bq = (be1 @ Wq[hsel].astype(np.float64).transpose(1, 0, 2)
              .reshape(C, NHL * HD)).astype(f32)
        bk = (be1 @ Wk[hsel].astype(np.float64).transpose(1, 0, 2)
              .reshape(C, NHL * HD)).astype(f32)
        bvv = (be1 @ Wv[hsel].astype(np.float64).transpose(1, 0, 2)
               .reshape(C, NHL * HD)).astype(f32)
        bqk_arr = np.stack([bq[0:P], bq[P:2 * P], bk[0:P], bk[P:2 * P]],
                           axis=1).astype(f32)
        wo_arr = np.ascontiguousarray(
            Wo[NHL * HD * r: NHL * HD * (r + 1)].reshape(2, P, C)
            .transpose(1, 0, 2)).astype(NPDT)
        lidx = (np.arange(TLOC) // P) * 512 + P * r + (np.arange(TLOC) % P)
        in_maps.append({
            "xh": x[b].astype(NPDT),
            "xres": np.ascontiguousarray(x[b, lidx] + bo[None, :]),
            "wq": wq_arr, "wk": wk_arr, "wv": wv_arr,
            "bqk": bqk_arr, "bv": bvv.reshape(1, NHL * HD),
            "wo": wo_arr, "bo": bo_arr,
            "w1": w1_arr, "b1p": b1_arr, "w2": w2_arr, "b2": b2_arr,
        })
    return in_maps


def kernel(**inputs):
    global _CACHED_NC, LAST_RESULTS
    if _CACHED_NC is None:
        _CACHED_NC = _build_module()
    in_maps = _prep_inputs(**inputs)
    res = bass_utils.run_bass_kernel_spmd(
        _CACHED_NC, in_maps, core_ids=list(range(NCORES)))
    LAST_RESULTS = res
    y = np.empty((B, T, C), np.float32)
    lidx0 = (np.arange(TLOC) // P) * 512 + (np.arange(TLOC) % P)
    for core in range(NCORES):
        b, r = divmod(core, TPG)
        y[b, lidx0 + P * r] = res.results[core]["y"]
    return y


# revision 16
# speedup vs baseline: 1.0772x; 1.0772x over previous
"""Trainium2 Bass kernel for a dense transformer block (B=2, T=2048, C=1024,
NH=16, HD=64, FF=4x), distributed over 8 NeuronCores.

Sharding: data-parallel over batch (2 groups of 4 cores) x tensor-parallel over
heads within a group (4 heads/core, Megatron-style), with sequence-parallel FFN:
the attention output partials are ReduceScattered over the T axis inside each
group, so each core runs LN2+FFN on its own 512 rows and returns them. The host
assembles the full [B, T, C] output. Matmul operands are bf16 (full PE rate,
1 cycle/row incl. 64-partition score matmuls and stationary loads); LayerNorm
and softmax statistics plus all PSUM accumulation stay in fp32.

LN gains are folded into the weights host-side (exact algebra):
  xn = g*z + be  (z = (x-mean)/std)  =>  xn @ W = z @ (g*W) + be @ W
so the chip computes z only; the be@W terms become small per-column biases.

DMA queue assignment (each engine sequencer issues its own in-order queue):
  sync    - x tiles, xres residuals, y output
  scalar  - attention weights/biases (wq/wk/wv/wo/bqk/bv/b1/b2)
  vector  - ReduceScatter staging writes (producer is vector anyway)
  gpsimd  - ReduceScatter result reads + FFN weight streaming (idle engine,
            so FFN weights prefetch during late attention)
"""

import contextlib
import os
import sys
import types

import ml_dtypes
import numpy as np

# --- NTFF profile hook shim (tracing support; harmless when unused) ---------
def _install_ntff_hook_shim():
    if "antenv.axon_hooks" in sys.modules:
        return
    try:
        import antenv
        import trn_agent_boot.trn_boot as tb

        mod = types.ModuleType("antenv.axon_hooks")
        holder = [None]
        mod.set_axon_ntff_profile_hook = lambda h: holder.__setitem__(0, h)
        mod.get_axon_ntff_profile_hook = lambda: holder[0]
        sys.modules["antenv.axon_hooks"] = mod
        antenv.axon_hooks = mod
        if os.path.exists("/opt/axon/libaxon_pjrt.so"):
            mod.set_axon_ntff_profile_hook(
                tb._ntff_profile_via_ctypes("/opt/axon/libaxon_pjrt.so")
            )
    except Exception:
        pass


_install_ntff_hook_shim()

import concourse.bass as bass
import concourse.mybir as mybir
import concourse.tile as tile
from concourse import bacc
from concourse import bass_utils
from concourse.masks import make_identity

# Problem shape (hardcoded per contest rules).
B, T, C, NH, HD = 2, 2048, 1024, 16, 64
FF = 4 * C  # 4096
EPS = 1e-6
P = 128
NCORES = 8
TPG = 4            # cores per batch group
NHL = NH // TPG    # local heads per core = 4
TLOC = T // TPG    # rows per core after ReduceScatter = 512
KO = C // P        # 8 contraction chunks over C
NFT = FF // P      # 32 f-tiles
NTT = T // P       # 16 t-tiles
NTB = T // 512     # 4 t-blocks

DT = mybir.dt.bfloat16   # matmul operand dtype
F32 = mybir.dt.float32
NPDT = ml_dtypes.bfloat16
MASK_NEG = -30000.0

_CACHED_NC = None

# Results of the most recent hardware run (BassKernelResults); lets test
# harnesses read exec_time_ns when BASS_TRACE=1.
LAST_RESULTS = None


def _build_module():
    nc = bacc.Bacc("TRN2", target_bir_lowering=False, debug=False,
                   num_devices=NCORES)

    x_in = nc.dram_tensor("xh", [T, C], DT, kind="ExternalInput").ap()
    xres_in = nc.dram_tensor("xres", [TLOC, C], F32, kind="ExternalInput").ap()
    wq_in = nc.dram_tensor("wq", [P, KO, NHL * HD], DT, kind="ExternalInput").ap()
    wk_in = nc.dram_tensor("wk", [P, KO, NHL * HD], DT, kind="ExternalInput").ap()
    wv_in = nc.dram_tensor("wv", [P, KO, NHL * HD], DT, kind="ExternalInput").ap()
    bqk_in = nc.dram_tensor("bqk", [P, 4], F32, kind="ExternalInput").ap()
    bv_in = nc.dram_tensor("bv", [1, NHL * HD], F32, kind="ExternalInput").ap()
    wo_in = nc.dram_tensor("wo", [P, 2, C], DT, kind="ExternalInput").ap()
    bo_in = nc.dram_tensor("bo", [1, C], F32, kind="ExternalInput").ap()
    w1_in = nc.dram_tensor("w1", [P, NFT, KO, P], DT, kind="ExternalInput").ap()
    b1_in = nc.dram_tensor("b1p", [P, NFT], F32, kind="ExternalInput").ap()
    w2_in = nc.dram_tensor("w2", [P, NFT, C], DT, kind="ExternalInput").ap()
    b2_in = nc.dram_tensor("b2", [1, C], F32, kind="ExternalInput").ap()
    y_out = nc.dram_tensor("y", [TLOC, C], F32, kind="ExternalOutput").ap()

    with tile.TileContext(nc) as tc:
        _emit(nc, tc, x_in, xres_in, wq_in, wk_in, wv_in, bqk_in, bv_in,
              wo_in, bo_in, w1_in, b1_in, w2_in, b2_in, y_out)
    nc.compile()
    return nc


def _layernorm_z(nc, stats, xt, z_out):
    """z = (x - mean(x)) / unbiased_std(x), rows on partitions.

    Rsqrt keeps the scalar engine on a single activation table (the
    reference's +EPS on std is a 1e-6 relative difference, far below
    tolerance).  xt: [P, C] fp32 SBUF tile; z_out: [P, C] DT tile."""
    s6 = stats.tile([P, 2, 6], F32, tag="bn6")
    nc.vector.bn_stats(s6[:, 0, :], xt[:, 0:C // 2])
    nc.vector.bn_stats(s6[:, 1, :], xt[:, C // 2:C])
    mv = stats.tile([P, 2], F32, tag="bnmv")
    nc.vector.bn_aggr(mv[:], s6[:])
    std = stats.tile([P, 1], F32, tag="bnstd")
    nc.scalar.activation(std[:], mv[:, 1:2],
                         mybir.ActivationFunctionType.Sqrt,
                         scale=float(C) / float(C - 1))
    rstd = stats.tile([P, 1], F32, tag="bnrstd")
    nc.vector.reciprocal(rstd[:], std[:])
    nc.vector.tensor_scalar(z_out[:], xt[:], mv[:, 0:1], rstd[:],
                            mybir.AluOpType.subtract, mybir.AluOpType.mult)


def _emit(nc, tc, x_in, xres_in, wq_in, wk_in, wv_in, bqk_in, bv_in,
          wo_in, bo_in, w1_in, b1_in, w2_in, b2_in, y_out):
    ctx = contextlib.ExitStack()
    # persistent pools (whole kernel)
    fp = ctx.enter_context(tc.tile_pool(name="fixed", bufs=1))
    stats = ctx.enter_context(tc.tile_pool(name="stats", bufs=6))
    ztiles = ctx.enter_context(tc.tile_pool(name="ztiles", bufs=2))
    dram = ctx.enter_context(tc.tile_pool(name="dram", bufs=1, space="DRAM"))

    # --- persistent constants -----------------------------------------------
    ident = fp.tile([P, P], DT)
    make_identity(nc, ident[:])
    ones1 = fp.tile([1, P], F32)
    nc.vector.memset(ones1[:], 1.0)
    ones1r = fp.tile([1, P], DT)
    nc.vector.tensor_copy(ones1r[:], ones1[:])
    b1p = fp.tile([P, NFT], F32)
    b2 = fp.tile([1, C], F32)
    b2r = fp.tile([1, C], DT)
    w2keep = fp.tile([P, NFT, C], DT)

    rs_ins = [dram.tile([512, C], DT, name=f"rsin{j}") for j in range(NTB)]
    rs_outs = [dram.tile([P, C], DT, name=f"rsout{j}") for j in range(NTB)]

    # Tiny warmup ReduceScatter issued during P1: the first collective pays
    # ~25us of channel setup that would otherwise land on band 0's RS.
    warm_in = dram.tile([TPG, P], F32, name="warmin")
    warm_out = dram.tile([1, P], F32, name="warmout")
    wtile = fp.tile([TPG, P], F32)
    nc.vector.memset(wtile[:], 0.0)
    nc.sync.dma_start(warm_in[:], wtile[:])
    nc.gpsimd.collective_compute(
        "ReduceScatter", mybir.AluOpType.add,
        replica_groups=[[0, 1, 2, 3], [4, 5, 6, 7]],
        ins=[warm_in.opt()], outs=[warm_out.opt()],
    )

    # attention-scope pools: released after P4
    actx = contextlib.ExitStack()
    fpa = actx.enter_context(tc.tile_pool(name="fixeda", bufs=1))
    abig = actx.enter_context(tc.tile_pool(name="abig", bufs=1))

    zero512 = fpa.tile([P, 512], F32)
    nc.vector.memset(zero512[:], 0.0)
    masks = fpa.tile([P, 512], F32)
    # keep score where (t_rel - s_rel) >= 0 else MASK_NEG; diagonal blocks are
    # emitted in truncated coordinates so the k=0 mask serves all of them.
    nc.gpsimd.affine_select(
        out=masks[:], in_=zero512[:],
        compare_op=mybir.AluOpType.is_ge, fill=MASK_NEG,
        base=0, channel_multiplier=-1, pattern=[[1, 512]],
    )

    qctx = contextlib.ExitStack()
    qkvw = qctx.enter_context(tc.tile_pool(name="qkvw", bufs=1))
    # weight/constant DMAs ride the scalar queue so the x tiles (sync queue)
    # land first; order matches first use on chip.
    bqk = qkvw.tile([P, 4], F32)
    nc.scalar.dma_start(bqk[:], bqk_in[:])
    bv = qkvw.tile([1, NHL * HD], F32)
    nc.scalar.dma_start(bv[:], bv_in[:])
    nc.scalar.dma_start(b1p[:], b1_in[:])
    nc.scalar.dma_start(b2[:], b2_in[:])
    nc.vector.tensor_copy(b2r[:], b2[:])
    bv_r = qkvw.tile([1, NHL * HD], DT)
    nc.vector.tensor_copy(bv_r[:], bv[:])
    # wq/wk/wv/wo DMAs are issued from inside the P1 loop (below) so the
    # first x tiles own the DMA rings at startup.
    wq = qkvw.tile([P, KO, NHL * HD], DT)
    wk = qkvw.tile([P, KO, NHL * HD], DT)
    wv = qkvw.tile([P, KO, NHL * HD], DT)
    wo = fpa.tile([P, 2, C], DT)

    qT2 = abig.tile([P, 2, T], DT)
    kT2 = abig.tile([P, 2, T], DT)
    v_sb = abig.tile([P, NTT, NHL, HD + 1], DT)
    ones_c = fpa.tile([P, 1], F32)
    nc.vector.memset(ones_c[:], 1.0)
    # esel: [33, P] selector; row 0 -> out partitions 0:64, row 32 ->
    # 64:128 (SBUF partition bases must be 0/32/64, so a Z pair lives on
    # rows 0 and 32).  One PE op broadcasts both 1/Z rows of a head pair.
    esel = fpa.tile([33, P], DT)
    nc.vector.memset(esel[:], 0.0)
    nc.vector.memset(esel[0:1, 0:HD], 1.0)
    nc.vector.memset(esel[32:33, HD:P], 1.0)
    nc.vector.tensor_copy(
        v_sb[:, :, :, HD:HD + 1],
        ones_c[:, :, None, None].to_broadcast((P, NTT, NHL, 1)))

    # --- P1+P2: per 512-row block: LN1, transpose, QKV ----------------------
    with tc.tile_pool(name="xnTp", bufs=2) as xnTp, \
         tc.tile_pool(name="xtiles", bufs=6) as xtiles, \
         tc.tile_pool(name="pst1", bufs=3, space="PSUM") as pst1, \
         tc.tile_pool(name="psqk", bufs=4, space="PSUM") as psqk:
        for tb in range(NTB):
            xnT = xnTp.tile([P, KO, 512], DT, tag="xnT")
            for tt4 in range(4):
                tt = 4 * tb + tt4
                xt = xtiles.tile([P, C], DT, tag="x")
                nc.sync.dma_start(xt[:, 0:C // 2],
                                  x_in[tt * P:(tt + 1) * P, 0:C // 2])
                nc.sync.dma_start(xt[:, C // 2:C],
                                  x_in[tt * P:(tt + 1) * P, C // 2:C])
                z = ztiles.tile([P, C], DT, tag="z")
                _layernorm_z(nc, stats, xt, z)
                if tt == 1:
                    nc.scalar.dma_start(wq[:], wq_in[:])
                elif tt == 2:
                    nc.scalar.dma_start(wk[:], wk_in[:])
                elif tt == 3:
                    nc.scalar.dma_start(wv[:], wv_in[:])
                elif tt == 5:
                    nc.scalar.dma_start(wo[:], wo_in[:])
                for cg in range(2):
                    tp = pst1.tile([P, 512], DT, tag="tp")
                    for c4 in range(4):
                        ci = 4 * cg + c4
                        nc.tensor.transpose(tp[:, c4 * P:(c4 + 1) * P],
                                            z[:, ci * P:(ci + 1) * P],
                                            ident[:])
                    nc.vector.tensor_copy(
                        xnT[:, 4 * cg:4 * cg + 4, tt4 * P:(tt4 + 1) * P],
                        tp.rearrange("p (c t) -> p c t", c=4))
            for pp in range(2):
                for dst, w, bcol in ((qT2, wq, pp), (kT2, wk, 2 + pp)):
                    ps = psqk.tile([P, 512], F32, tag="qk")
                    for ko in range(KO):
                        nc.tensor.matmul(
                            ps[:], w[:, ko, pp * P:(pp + 1) * P],
                            xnT[:, ko, :],
                            start=(ko == 0), stop=(ko == KO - 1))
                    nc.vector.tensor_scalar_add(
                        dst[:, pp, tb * 512:(tb + 1) * 512], ps[:],
                        bqk[:, bcol:bcol + 1])
            for tt4 in range(4):
                tt = 4 * tb + tt4
                ps = psqk.tile([P, NHL * HD], F32, tag="qk")
                nc.tensor.matmul(ps[:, 0:NHL * HD], ones1r[0:1, :],
                                 bv_r[0:1, :],
                                 start=True, stop=False, skip_group_check=True)
                for ko in range(KO):
                    nc.tensor.matmul(
                        ps[:, 0:NHL * HD],
                        xnT[:, ko, tt4 * P:(tt4 + 1) * P], wv[:, ko, :],
                        start=False, stop=(ko == KO - 1),
                        skip_group_check=True)
                for h in range(NHL):
                    nc.vector.tensor_copy(v_sb[:, tt, h, 0:HD],
                                          ps[:, h * HD:(h + 1) * HD])
    qctx.close()

    # --- P3..P7 ---------------------------------------------------------------
    x2 = ctx.enter_context(tc.tile_pool(name="x2p", bufs=1, side="right")).tile(
        [P, TLOC // P, C], F32)
    xn2T = ctx.enter_context(
        tc.tile_pool(name="xn2Tp", bufs=1, side="right")).tile(
        [P, KO, TLOC], DT)

    with tc.tile_pool(name="ptp", bufs=4) as ptp, \
         tc.tile_pool(name="mskp", bufs=2) as mskp, \
         tc.tile_pool(name="rzp", bufs=3) as rzp, \
         tc.tile_pool(name="bandp", bufs=2) as bandp, \
         tc.tile_pool(name="rstage", bufs=2) as rstage, \
         tc.tile_pool(name="p6", bufs=2) as p6, \
         tc.tile_pool(name="pst2", bufs=1, space="PSUM") as pst2:

        def emit_p6(j):
            """x2 band j = rs_out_j + (xres+bo); LN2; transpose into xn2T."""
            # rs_out depends on the collective: keep its DMA on the gpsimd
            # queue so no compute queue head-of-line blocks on it.
            x2b = p6.tile([P, C], DT, tag="x2b")
            nc.scalar.dma_start(x2b[:], rs_outs[j][:])
            xrt = p6.tile([P, C], F32, tag="xrt")
            nc.sync.dma_start(xrt[:], xres_in[j * P:(j + 1) * P, :])
            nc.vector.tensor_add(x2[:, j, :], x2b[:], xrt[:])
            z2 = ztiles.tile([P, C], DT, tag="z")
            _layernorm_z(nc, stats, x2[:, j, :], z2)
            for cg in range(2):
                tp = pst2.tile([P, 512], DT, tag="tp2")
                for c4 in range(4):
                    ci = 4 * cg + c4
                    nc.tensor.transpose(tp[:, c4 * P:(c4 + 1) * P],
                                        z2[:, ci * P:(ci + 1) * P],
                                        ident[:])
                nc.vector.tensor_copy(
                    xn2T[:, 4 * cg:4 * cg + 4, j * P:(j + 1) * P],
                    tp.rearrange("p (c t) -> p c t", c=4))

        with tc.tile_pool(name="psc", bufs=2, space="PSUM") as pssc, \
             tc.tile_pool(name="psctx", bufs=3, space="PSUM") as psctx, \
             tc.tile_pool(name="pso", bufs=2, space="PSUM") as pso:

            def emit_wo(j, ctxb):
                """Wo partials for band j -> rs_in_j, then its own RS."""
                for tt4 in range(4):
                    stg = rstage.tile([P, C], DT, tag="stg")
                    for cb in range(2):
                        ops_ = pso.tile([P, 512], F32, tag="o")
                        for ch in range(2):
                            nc.tensor.matmul(
                                ops_[:],
                                ctxb[:, ch, tt4 * P:(tt4 + 1) * P],
                                wo[:, ch, cb * 512:(cb + 1) * 512],
                                start=(ch == 0), stop=(ch == 1))
                        nc.vector.tensor_copy(stg[:, cb * 512:(cb + 1) * 512],
                                              ops_[:])
                    nc.scalar.dma_start(rs_ins[j][tt4 * P:(tt4 + 1) * P, :],
                                        stg[:])
                nc.gpsimd.collective_compute(
                    "ReduceScatter", mybir.AluOpType.add,
                    replica_groups=[[0, 1, 2, 3], [4, 5, 6, 7]],
                    ins=[rs_ins[j].opt()], outs=[rs_outs[j].opt()],
                )

            # Bands run in DESCENDING order: band 3 (the largest) finishes
            # first so its ReduceScatter lands long before the FFN needs it;
            # band 0's RS is covered by the first FFN half (bands 2,3).
            pend_wo = None
            for bi, j in enumerate([3, 2, 1, 0]):
                ns = 4 * (j + 1)
                ctxb = bandp.tile([P, 2, 512], DT, tag="ctxb")
                # 1/Z staging: Z rows gathered per head pair, reciprocal
                # batched over 2 partitions so the serial [_,512] sweep is
                # paid twice per band, not 4x.  esel broadcasts both rows of
                # a pair to 64 partitions each in one PE op.
                zrp = [rzp.tile([33, 512], F32, tag=f"zr{p}", name=f"zr{p}")
                       for p in range(2)]
                rzp_ = [rzp.tile([33, 512], DT, tag=f"rz{p}", name=f"rz{p}")
                        for p in range(2)]
                for p in range(2):
                    # rows 1..31 are never written; keep them finite so the
                    # batched reciprocal and the esel matmul stay clean.
                    nc.vector.memset(zrp[p][:], 1.0)

                def norm_pair(pp, cps01):
                    """PE broadcast of a pair's 1/Z + fused psum->sbuf
                    scale into ctxb[:, pp, :]."""
                    zbp = pso.tile([P, 512], F32, tag="o")
                    nc.tensor.matmul(zbp[:], esel[:], rzp_[pp][:],
                                     start=True, stop=True)
                    zb = rzp.tile([P, 512], F32, tag="zb")
                    nc.vector.tensor_copy(zb[:], zbp[:])
                    for sub in range(2):
                        nc.vector.tensor_tensor(
                            ctxb[sub * HD:(sub + 1) * HD, pp, :],
                            cps01[sub][0:HD, :], zb[sub * HD:(sub + 1) * HD, :],
                            mybir.AluOpType.mult)

                norm_q = []
                for h in range(NHL):
                    pp, poff = h // 2, 64 * (h % 2)
                    cps = psctx.tile([HD + 1, 512], F32, tag="ctx")
                    for i in range(ns):
                        k = i - 4 * j
                        off = max(k, 0) * P       # truncated query offset
                        W = 512 - off
                        sps = pssc.tile([P, 512], F32, tag="sc")
                        nc.tensor.matmul(
                            sps[:, 0:W],
                            kT2[poff:poff + HD, pp, i * P:(i + 1) * P],
                            qT2[poff:poff + HD, pp,
                                j * 512 + off:(j + 1) * 512],
                            start=True, stop=True)
                        pT = ptp.tile([P, 512], DT, tag="pT")
                        if k >= 0:
                            ms = mskp.tile([P, 512], DT, tag="ms")
                            nc.vector.scalar_tensor_tensor(
                                ms[:, 0:W], sps[:, 0:W], 0.125, masks[:, 0:W],
                                mybir.AluOpType.mult, mybir.AluOpType.add)
                            nc.scalar.activation(
                                pT[:, 0:W], ms[:, 0:W],
                                mybir.ActivationFunctionType.Exp)
                            # region [off:off+128] sees its last key block
                            # here; later diagonals only touch columns right
                            # of it, so close its accumulation now.
                            nc.tensor.matmul(cps[:, off:off + P],
                                             v_sb[:, i, h, :], pT[:, 0:P],
                                             start=(i == 0), stop=True,
                                             skip_group_check=True)
                            if k < 3:
                                nc.tensor.matmul(cps[:, off + P:512],
                                                 v_sb[:, i, h, :],
                                                 pT[:, P:W],
                                                 start=(i == 0), stop=False,
                                                 skip_group_check=True)
                        else:
                            nc.scalar.activation(
                                pT[:], sps[:],
                                mybir.ActivationFunctionType.Exp, scale=0.125)
                            nc.tensor.matmul(cps[:], v_sb[:, i, h, :], pT[:],
                                             start=(i == 0), stop=False,
                                             skip_group_check=True)
                    zrow = 32 * (h % 2)
                    nc.vector.tensor_copy(zrp[h // 2][zrow:zrow + 1, :],
                                          cps[HD:HD + 1, :])
                    if h % 2 == 1:
                        with nc.allow_low_precision(
                                reason="1/Z rounded to bf16 for PE broadcast"):
                            nc.vector.reciprocal(rzp_[h // 2][:],
                                                 zrp[h // 2][:])
                    norm_q.append(cps)
                    if h == 0 and pend_wo is not None:
                        emit_wo(*pend_wo)
                        pend_wo = None
                    if h == 2:
                        norm_pair(0, norm_q[0:2])
                norm_pair(1, norm_q[2:4])

                if bi < NTB - 1:
                    pend_wo = (j, ctxb)
                else:
                    emit_wo(j, ctxb)
                if j == 2:
                    # RS3 has landed by the time the queues get here.
                    emit_p6(3)
                    # W2 prefetch: issue the whole stream now (no pool
                    # throttling) so it's resident before the FFN starts.
                    for ft in range(NFT):
                        nc.sync.dma_start(w2keep[:, ft, :], w2_in[:, ft, :])
                elif j == 1:
                    emit_p6(2)
            emit_p6(1)

        # --- P7: FFN; th half 0 first (bands 0,1), band-3 P6 in between ------
        # W2 is loaded once into SBUF (w2keep) during half 0 and reused in
        # half 1; W1 streams per half on the gpsimd queue, which at emission
        # time sits just after the band-2 rs_out read, so half-0 weights
        # prefetch during band-3 attention compute.
        with tc.tile_pool(name="w1p", bufs=6) as w1p, \
             tc.tile_pool(name="rp", bufs=2) as rp, \
             tc.tile_pool(name="psh1", bufs=3, space="PSUM") as psh1, \
             tc.tile_pool(name="psh2", bufs=4, space="PSUM") as psh2, \
             tc.tile_pool(name="yp", bufs=2) as yp:

            def ffn_half(th, mid=None):
                h2ps = []
                for tt2 in range(2):
                    for cb in range(2):
                        hp = psh2.tile([P, 512], F32, tag="h2")
                        nc.tensor.matmul(hp[:], ones1r[0:1, :],
                                         b2r[0:1, cb * 512:(cb + 1) * 512],
                                         start=True, stop=False,
                                         skip_group_check=True)
                        h2ps.append(hp)
                for ft in range(NFT):
                    if ft == 16 and mid is not None:
                        mid()
                    w1t = w1p.tile([P, KO, P], DT, tag="w1")
                    nc.sync.dma_start(w1t[:], w1_in[:, ft, :, :])
                    h1 = psh1.tile([P, 256], F32, tag="h1")
                    for ko in range(KO):
                        nc.tensor.matmul(h1[:], w1t[:, ko, :],
                                         xn2T[:, ko, th * 256:(th + 1) * 256],
                                         start=(ko == 0), stop=(ko == KO - 1))
                    rT = rp.tile([P, 256], DT, tag="rT")
                    nc.vector.tensor_scalar(rT[:], h1[:], b1p[:, ft:ft + 1],
                                            0.0, mybir.AluOpType.add,
                                            mybir.AluOpType.max)
                    for tt2 in range(2):
                        for cb in range(2):
                            nc.tensor.matmul(
                                h2ps[2 * tt2 + cb][:],
                                rT[:, tt2 * P:(tt2 + 1) * P],
                                w2keep[:, ft, cb * 512:(cb + 1) * 512],
                                start=False, stop=(ft == NFT - 1),
                                skip_group_check=True)
                for tt2 in range(2):
                    gt = 2 * th + tt2
                    for cb in range(2):
                        yt = yp.tile([P, 512], F32, tag="y")
                        nc.vector.scalar_tensor_tensor(
                            yt[:], h2ps[2 * tt2 + cb][:], 1.0,
                            x2[:, gt, cb * 512:(cb + 1) * 512],
                            mybir.AluOpType.mult, mybir.AluOpType.add)
                        nc.sync.dma_start(
                            y_out[gt * P:(gt + 1) * P,
                                  cb * 512:(cb + 1) * 512],
                            yt[:])

            ffn_half(1, mid=lambda: emit_p6(0))
            ffn_half(0)

    actx.close()
    ctx.close()


def _prep_inputs(x, Wq, Wk, Wv, Wo, bo, W1, b1, W2, b2, g1, be1, g2, be2):
    """Host-side sharding + layout packing. Returns list of 8 in_maps."""
    f32 = np.float32
    x = np.asarray(x, f32)
    Wq, Wk, Wv = (np.asarray(a, f32) for a in (Wq, Wk, Wv))
    Wo, bo = np.asarray(Wo, f32), np.asarray(bo, f32)
    W1, b1, W2, b2 = (np.asarray(a, f32) for a in (W1, b1, W2, b2))
    g1, be1, g2, be2 = (np.asarray(a, np.float64) for a in (g1, be1, g2, be2))

    def pack_qkv(W):  # [NHL, C, HD] g-folded -> [P, KO, NHL*HD]
        Wl = (g1[None, :, None] * W.astype(np.float64)).astype(f32)
        flat = Wl.transpose(1, 0, 2).reshape(C, NHL * HD)   # [c, col]
        return np.ascontiguousarray(flat.reshape(KO, P, NHL * HD)
                                    .transpose(1, 0, 2)).astype(NPDT)

    # W1 folded with g2: [C, FF] -> [P, NFT, KO, P]
    W1f = (g2[:, None] * W1.astype(np.float64)).astype(f32)
    w1_arr = np.ascontiguousarray(
        W1f.reshape(KO, P, NFT, P).transpose(1, 2, 0, 3)).astype(NPDT)
    b1p = (b1.astype(np.float64) + be2 @ W1.astype(np.float64)).astype(f32)
    b1_arr = np.ascontiguousarray(b1p.reshape(NFT, P).T)
    w2_arr = np.ascontiguousarray(
        W2.reshape(NFT, P, C).transpose(1, 0, 2)).astype(NPDT)
    b2_arr = b2.reshape(1, C)
    bo_arr = bo.reshape(1, C)

    in_maps = []
    for core in range(NCORES):
        b, r = divmod(core, TPG)
        hsel = slice(NHL * r, NHL * (r + 1))
        wq_arr = pack_qkv(Wq[hsel])
        wk_arr = pack_qkv(Wk[hsel])
        wv_arr = pack_qkv(Wv[hsel])
        # be1-induced biases (exact): col order = head-major within 256
        bq = (be1 @ Wq[hsel].astype(np.float64).transpose(1, 0, 2)
              .reshape(C, NHL * HD)).astype(f32)
        bk = (be1 @ Wk[hsel].astype(np.float64).transpose(1, 0, 2)
              .reshape(C, NHL * HD)).astype(f32)
        bvv = (be1 @ Wv[hsel].astype(np.float64).transpose(1, 0, 2)
               .reshape(C, NHL * HD)).astype(f32)
        bqk_arr = np.stack([bq[0:P], bq[P:2 * P], bk[0:P], bk[P:2 * P]],
                           axis=1).astype(f32)
        wo_arr = np.ascontiguousarray(
            Wo[NHL * HD * r: NHL * HD * (r + 1)].reshape(2, P, C)
            .transpose(1, 0, 2)).astype(NPDT)
        lidx = (np.arange(TLOC) // P) * 512 + P * r + (np.arange(TLOC) % P)
        in_maps.append({
            "xh": x[b].astype(NPDT),
            "xres": np.ascontiguousarray(x[b, lidx] + bo[None, :]),
            "wq": wq_arr, "wk": wk_arr, "wv": wv_arr,
            "bqk": bqk_arr, "bv": bvv.reshape(1, NHL * HD),
            "wo": wo_arr, "bo": bo_arr,
            "w1": w1_arr, "b1p": b1_arr, "w2": w2_arr, "b2": b2_arr,
        })
    return in_maps


def kernel(**inputs):
    global _CACHED_NC, LAST_RESULTS
    if _CACHED_NC is None:
        _CACHED_NC = _build_module()
    in_maps = _prep_inputs(**inputs)
    res = bass_utils.run_bass_kernel_spmd(
        _CACHED_NC, in_maps, core_ids=list(range(NCORES)))
    LAST_RESULTS = res
    y = np.empty((B, T, C), np.float32)
    lidx0 = (np.arange(TLOC) // P) * 512 + (np.arange(TLOC) % P)
    for core in range(NCORES):
        b, r = divmod(core, TPG)
        y[b, lidx0 + P * r] = res.results[core]["y"]
    return y
